# revision 65
# baseline (speedup 1.0000x reference)
"""Causal self-attention TRN2 Bass kernel (bf16, software-pipelined).

Sharding: 8 cores = 4 batches x 2 head-groups. Core c handles batch c//2 and
heads (c%2)*8 .. (c%2)*8+8 (of 16). Each core computes its heads' attention
and a partial output projection; the host sums the two partials per batch and
adds b_out.

Design notes:
  - all matmul operands bf16 (FWL weight loads overlap the stream; f32r
    self-loading matmuls serialize a ~180ns weight load per matmul)
  - DRAM inputs are host-packed into small fine-grained tiles so the first
    S matmul only waits on ~1.5MB (xT is token-chunk-major, weights per-ft)
  - single pool scope, one long instruction stream: QK/V/out projections are
    emitted as deadline-scheduled "filler" half-chains interleaved into the
    attention cadence, so the PE never idles
  - causal mask applied by zeroing exp(S) tiles (affine_select on gpsimd/
    vector), not by adding -inf into PSUM: keeps DVE off the S->exp path
  - softmax denominator via the ones-column of V (row 64 of the PV PSUM);
    normalization split: PSUM drain (copies) immediate, recip/broadcast/mult
    deferred into later steps as filler DVE/gpsimd work
  - warmup matmuls ramp the PE p-state while input DMAs stream

Layouts on chip (per core):
  XTC   4 x [128, 8x512] bf16  x[b].T token-chunk-major: chunk c, d-chunk l
  WQF/WKF 4 x [128, 8x128] bf16 per-ft Q/K weights; WV 2 x [128, 4x512]
  WOH   2 x [128, 4x512] bf16
  QKT   8 x [128, 2048] bf16  Q^T (0..3) / K^T (4..7) features x tokens
  V     16 x [128, 520] bf16  tokens x (8 heads x (64 vals + ones col))
  e     [128, 1024] bf16      exp(S^T) per k-tile, both heads
  AOT   4 x [128, 2048] bf16  normalized attention out (features x tokens)
  y     [2048, 1024] bf16     partial output projection
"""
import sys

sys.path.insert(0, "/opt/trn_rl_repo")

import numpy as np
import ml_dtypes

D_MODEL = 1024
N_HEADS = 16
B = 4
T = 2048
HD = 64
N_CORES = 8
NH_LOC = N_HEADS // 2  # heads per core
FQ = NH_LOC * HD  # 512 local features

_prog_cache = {}


def build_program(tok=T, debug_dumps=False):
    """Build the single-core SPMD Bass program. tok must be a multiple of 512."""
    import concourse.mybir as mybir
    import concourse.tile as tile
    from concourse import bacc

    f32 = mybir.dt.float32
    bf16 = mybir.dt.bfloat16
    P = 128
    QC = 512  # q-chunk width
    KC = D_MODEL // P  # 8 d-model chunks
    TT = tok // P  # token tiles
    NJ = tok // QC  # q-chunks
    NDC = FQ // P  # 4 feature chunks

    nc = bacc.Bacc("TRN2", target_bir_lowering=False, debug=False, num_devices=N_CORES)

    # fine-grained DRAM inputs (host-packed); one tensor per DMA piece so
    # tile-granular dependencies stay small
    xTc = [
        nc.dram_tensor(f"xT{c}", [P, KC * QC], bf16, kind="ExternalInput")
        for c in range(NJ)
    ]
    wqf = [
        nc.dram_tensor(f"wq{ft}", [P, KC * P], bf16, kind="ExternalInput")
        for ft in range(NDC)
    ]
    wkf = [
        nc.dram_tensor(f"wk{ft}", [P, KC * P], bf16, kind="ExternalInput")
        for ft in range(NDC)
    ]
    wvh = [
        nc.dram_tensor(f"wv{h}", [P, 4 * FQ], bf16, kind="ExternalInput")
        for h in range(2)
    ]
    woh = [
        nc.dram_tensor(f"wo{h}", [P, NDC * QC], bf16, kind="ExternalInput")
        for h in range(2)
    ]
    y = nc.dram_tensor("y", [tok, D_MODEL], bf16, kind="ExternalOutput")

    with tile.TileContext(nc) as tc:
        with (
            tc.tile_pool(name="wp", bufs=1) as wp,
            tc.tile_pool(name="xtp", bufs=1) as xtp,
            tc.tile_pool(name="qktp", bufs=1) as qktp,
            tc.tile_pool(name="vp", bufs=1) as vp,
            tc.tile_pool(name="aotp", bufs=1) as aotp,
            tc.tile_pool(name="ep", bufs=3) as ep,
            tc.tile_pool(name="ystp", bufs=4) as ystp,
            tc.tile_pool(name="ybp", bufs=1) as ybp,
            tc.tile_pool(name="mvp", bufs=1) as mvp,
            tc.tile_pool(name="nrm", bufs=6) as nrmp,
            tc.tile_pool(name="nrs", bufs=6) as nrsp,
            tc.tile_pool(name="big", bufs=2, space="PSUM") as bigp,   # 4 banks
            tc.tile_pool(name="pvp", bufs=2, space="PSUM") as pvp,    # 2 banks
            tc.tile_pool(name="prj", bufs=2, space="PSUM") as prjp,   # 2 banks
        ):
            XTC = [wp.tile([P, KC * QC], bf16, tag=f"xtc{c}", name=f"xtc{c}") for c in range(NJ)]
            WQF = [wp.tile([P, KC * P], bf16, tag=f"wqf{ft}", name=f"wqf{ft}") for ft in range(NDC)]
            WKF = [wp.tile([P, KC * P], bf16, tag=f"wkf{ft}", name=f"wkf{ft}") for ft in range(NDC)]
            WVH = [wp.tile([P, 4 * FQ], bf16, tag=f"wvh{h}", name=f"wvh{h}") for h in range(2)]
            WOH = [wp.tile([P, NDC * QC], bf16, tag=f"woh{h}", name=f"woh{h}") for h in range(2)]
            QKT = [qktp.tile([P, tok], bf16, tag=f"qkt{i}", name=f"qkt{i}") for i in range(8)]
            V = [vp.tile([P, NH_LOC * (HD + 1)], bf16, tag=f"v{i}", name=f"v{i}") for i in range(TT)]
            AOT = [aotp.tile([P, tok], bf16, tag=f"aot{d}", name=f"aot{d}") for d in range(NDC)]

            # views
            def xt_view(l, c):  # [128, 512] of d-chunk l, token chunk c
                return XTC[c][:, l * QC : (l + 1) * QC]

            def wq_view(ft, l):
                return WQF[ft][:, l * P : (l + 1) * P]

            def wk_view(ft, l):
                return WKF[ft][:, l * P : (l + 1) * P]

            def wv_view(l):  # [128, 512] all 8 heads' V features, d-chunk l
                return WVH[l // 4][:, (l % 4) * FQ : (l % 4 + 1) * FQ]

            def wo_view(h, d):
                return WOH[h][:, d * QC : (d + 1) * QC]

            # ---------------- input DMAs ----------------
            # three issue queues in parallel, DMA-bandwidth-ordered: the
            # first-S critical set (xtc0 + wq_ft0 + wk_ft0, ~1.5MB) leads,
            # split so no queue serialises more than its share. The xtc
            # pieces are halved (l 0-3 / l 4-7) so region-granular deps let
            # half-chains start as soon as their half lands.
            HX = KC * QC // 2
            nc.gpsimd.dma_start(out=XTC[0][:, :HX], in_=xTc[0][:, :HX])
            nc.sync.dma_start(out=XTC[0][:, HX:], in_=xTc[0][:, HX:])
            nc.scalar.dma_start(out=WQF[0][:], in_=wqf[0][:])
            nc.scalar.dma_start(out=WKF[0][:], in_=wkf[0][:])
            nc.gpsimd.dma_start(out=WVH[0][:], in_=wvh[0][:])
            nc.sync.dma_start(out=WVH[1][:], in_=wvh[1][:])
            nc.gpsimd.dma_start(out=XTC[1][:, :HX], in_=xTc[1][:, :HX])
            nc.sync.dma_start(out=XTC[1][:, HX:], in_=xTc[1][:, HX:])
            nc.scalar.dma_start(out=WQF[1][:], in_=wqf[1][:])
            nc.scalar.dma_start(out=WKF[1][:], in_=wkf[1][:])
            nc.gpsimd.dma_start(out=XTC[2][:, :HX], in_=xTc[2][:, :HX])
            nc.sync.dma_start(out=XTC[2][:, HX:], in_=xTc[2][:, HX:])
            nc.gpsimd.dma_start(out=XTC[3][:, :HX], in_=xTc[3][:, :HX])
            nc.sync.dma_start(out=XTC[3][:, HX:], in_=xTc[3][:, HX:])
            nc.scalar.dma_start(out=WQF[2][:], in_=wqf[2][:])
            nc.scalar.dma_start(out=WKF[2][:], in_=wkf[2][:])
            nc.scalar.dma_start(out=WQF[3][:], in_=wqf[3][:])
            nc.scalar.dma_start(out=WKF[3][:], in_=wkf[3][:])
            nc.sync.dma_start(out=WOH[0][:], in_=woh[0][:])
            nc.sync.dma_start(out=WOH[1][:], in_=woh[1][:])

            # warm the exp table while input DMAs stream
            warm = mvp.tile([1, 8], f32, tag="warm", name="warm")
            nc.vector.memset(warm[:], 0.0)
            nc.scalar.activation(warm[:], warm[:], mybir.ActivationFunctionType.Exp)

            # ones columns of V (value cols are written by the projection
            # eviction; only col 64 of each head group needs initialising).
            # On the vector queue: gpsimd is busy issuing DMAs and must be
            # free early for the first diagonal ezeros.
            for tt in range(TT):
                vv = V[tt][:].rearrange("p (u c) -> p u c", c=HD + 1)
                nc.vector.memset(vv[:, :, HD : HD + 1], 1.0)

            # PE p-state warmup: dummy matmuls with no DMA deps keep the PE
            # "continuously executing" so real matmuls start at full clock
            dwa = mvp.tile([P, P], bf16, tag="dwa", name="dwa")
            dwb = mvp.tile([P, QC], bf16, tag="dwb", name="dwb")
            nc.vector.memset(dwa[:], 0.0)
            nc.vector.memset(dwb[:], 0.0)
            pwarm = prjp.tile([P, QC], f32, tag="prj", name="pwarm")
            for _ in range(16):
                nc.tensor.matmul(pwarm[:, :256], dwa[:], dwb[:, :256], start=True, stop=True)
            # ldweights-only extension: keeps the busy-streak alive through
            # the tail of the input-DMA wait without tying up a PSUM buffer,
            # so the first real matmuls start at the full 2.4GHz p-state
            for _ in range(40):
                nc.tensor.ldweights(dwa[:])

            # ---------------- filler chains (emitted in halves) ----------------
            open_chains = {}

            def qk_half(ft, c, part):
                """QKT[ft][:, c-chunk] = (w-slice)^T @ XT over l; 2 halves."""
                wv_ = wq_view if ft < 4 else wk_view
                fi = ft % 4
                key = ("qk", ft, c)
                if part == 0:
                    open_chains[key] = prjp.tile([P, QC], f32, tag="prj", name=f"pqk{ft}_{c}")
                p = open_chains[key]
                for l in range(4 * part, 4 * part + 4):
                    nc.tensor.matmul(
                        p[:],
                        wv_(fi, l),
                        xt_view(l, c),
                        start=(l == 0),
                        stop=(l == KC - 1),
                    )
                if part == 1:
                    del open_chains[key]
                    nc.vector.tensor_copy(QKT[ft][:, c * QC : (c + 1) * QC], p[:])

            def v_half(tt, part):
                """V[tt] value cols = XT-slice^T @ WV; 2 halves."""
                key = ("v", tt)
                if part == 0:
                    open_chains[key] = prjp.tile([P, FQ], f32, tag="prj", name=f"pv{tt}")
                p = open_chains[key]
                c, s = tt // 4, tt % 4
                for l in range(4 * part, 4 * part + 4):
                    nc.tensor.matmul(
                        p[:],
                        xt_view(l, c)[:, s * P : (s + 1) * P],
                        wv_view(l),
                        start=(l == 0),
                        stop=(l == KC - 1),
                    )
                if part == 1:
                    del open_chains[key]
                    vdst = V[tt][:].rearrange("p (u c) -> p u c", c=HD + 1)[:, :, 0:HD]
                    vsrc = p[:].rearrange("p (u c) -> p u c", c=HD)
                    nc.vector.tensor_copy(vdst, vsrc)

            def out_chain(tt, h, evict=None, dq=None):
                """y[tt-tile, h-half] = AOT-slice^T @ WO, 4 MMs + copy + DMA."""
                p = prjp.tile([P, QC], f32, tag="prj", name=f"py{tt}_{h}")
                for d in range(NDC):
                    nc.tensor.matmul(
                        p[:],
                        AOT[d][:, tt * P : (tt + 1) * P],
                        wo_view(h, d),
                        start=(d == 0),
                        stop=(d == NDC - 1),
                    )
                if evict is None:
                    evict = nc.vector.tensor_copy
                ysb = ystp.tile([P, QC], bf16, tag="y", name="ysb")
                evict(ysb[:], p[:])
                (dq or nc.sync).dma_start(
                    out=y[tt * P : (tt + 1) * P, h * QC : (h + 1) * QC],
                    in_=ysb[:],
                )

            # per-hp filler schedules: {step: [unit, ...]}.  hp0's schedule is
            # deadline-driven (chunk c is read by attention from j=c onward at
            # step 4*c*(c+1)/2...; V[tt] is read by PV at the k-step for tile
            # tt of each j >= tt//4).
            sched = {hp: {} for hp in range(4)}

            def put(hp, step, fn):
                sched[hp].setdefault(step, []).append(fn)

            # hp0, deadline-driven: QK c1 at steps 0-3 (read from step 4),
            # V4-7 at 4-7 (read 8-11), QK c2 at 8-11 (read 12), V8-11 at
            # 12-15 (read 16-19), QK c3 at 16-19 (read 24), V12-15 at 20-23
            # (read 28-31)
            for k, (ft, c) in enumerate([(0, 1), (4, 1)]):
                for part in (0, 1):
                    put(0, 2 * k + part, lambda ft=ft, c=c, part=part: qk_half(ft, c, part))
            for tt in (4, 5, 6, 7):
                for part in (0, 1):
                    put(0, tt, lambda tt=tt, part=part: v_half(tt, part))
            for k, (ft, c) in enumerate([(0, 2), (4, 2)]):
                for part in (0, 1):
                    put(0, 8 + 2 * k + part, lambda ft=ft, c=c, part=part: qk_half(ft, c, part))
            for tt in (8, 9, 10, 11):
                for part in (0, 1):
                    put(0, 4 + tt, lambda tt=tt, part=part: v_half(tt, part))
            for k, (ft, c) in enumerate([(0, 3), (4, 3)]):
                for part in (0, 1):
                    put(0, 16 + 2 * k + part, lambda ft=ft, c=c, part=part: qk_half(ft, c, part))
            for tt in (12, 13, 14, 15):
                for part in (0, 1):
                    put(0, 8 + tt, lambda tt=tt, part=part: v_half(tt, part))
            # QK(hp+1) spread over each hp's steps (hp0's start after its
            # deadline units; hp1/hp2 evenly — ~1 unit per 2.5 steps keeps
            # the per-step PE load under the exp cadence)
            for hp in range(3):
                base = 24 if hp == 0 else 0
                units = []
                for ft in (hp + 1, 4 + hp + 1):
                    for c in range(NJ):
                        for part in (0, 1):
                            units.append(
                                lambda ft=ft, c=c, part=part: qk_half(ft, c, part)
                            )
                # finish by ~80% of the hp's steps so the next head-pair's
                # first S never waits on the last chain's eviction
                span = max(len(units), int(0.8 * (40 - base)))
                for k, fn in enumerate(units):
                    put(hp, base + (k * span) // len(units), fn)

            # ---------------- pre-attention work ----------------
            # half-chain interleave: the QK c0 chains (gating the first S)
            # complete as early as the DMA pieces allow, V chains fill in
            qk_half(0, 0, 0)
            qk_half(4, 0, 0)
            qk_half(0, 0, 1)
            qk_half(4, 0, 1)
            for tt in range(4):
                for part in (0, 1):
                    v_half(tt, part)

            # ---------------- attention + interleaved fillers ----------------
            out_ready = []  # out-proj chains unlocked so far
            pending_fin = []  # deferred normalization finishers
            for hp in range(4):
                step = 0
                for j in range(NJ):
                    nkt = 4 * j + 4
                    pv = {
                        u: pvp.tile([HD + 1, QC], f32, tag="pv", name=f"pv{u}")
                        for u in (0, 1)
                    }
                    etiles = {}
                    sasd = {}

                    def emit_s_exp(i, j=j, etiles=etiles):
                        """S-pair + exp (+ causal zeroing) for k-tile i."""
                        s_ = i - 4 * j
                        w0 = 128 * s_ if s_ >= 0 else 0
                        st = bigp.tile([P, 2 * QC], f32, tag="big", name="st")
                        for u in (0, 1):
                            rs = slice(64 * u, 64 * u + 64)
                            nc.tensor.matmul(
                                st[:, u * QC + w0 : (u + 1) * QC],
                                QKT[4 + hp][rs, i * P : (i + 1) * P],
                                QKT[hp][rs, j * QC + w0 : (j + 1) * QC],
                                start=True,
                                stop=True,
                            )
                        win3 = st[:].rearrange("p (h q) -> p h q", h=2)
                        e = ep.tile([P, 2 * QC], bf16, tag="e", name="e")
                        e3 = e[:].rearrange("p (h q) -> p h q", h=2)

                        def ezero(u):
                            # causal mask: zero exp(S) where q < k in the
                            # diagonal 128x128 block (iota = q_hat - p >= 0
                            # keeps; else fill 0)
                            ev = e[:, u * QC + w0 : u * QC + w0 + P]
                            nc.gpsimd.affine_select(
                                out=ev,
                                in_=ev,
                                compare_op=mybir.AluOpType.is_ge,
                                fill=0.0,
                                base=0,
                                pattern=[[1, P]],
                                channel_multiplier=-1,
                            )

                        nc.scalar.activation(
                            e3[:, :, w0:QC],
                            win3[:, :, w0:QC],
                            mybir.ActivationFunctionType.Exp,
                            scale=0.125,
                        )
                        if s_ >= 0:
                            ezero(0)
                            ezero(1)
                        etiles[i] = e

                    emit_s_exp(0)
                    if nkt > 1:
                        emit_s_exp(1)
                    # p-state bridge: the first PV of a chunk can stall ~1us
                    # on the previous chunk's PSUM release; an idle PE drops
                    # to the 1.2GHz p-state and pays ~3us of half-speed ramp.
                    # No-dep dummy matmuls BEFORE the stalling PV keep the
                    # busy-streak alive through the wait.
                    nbub = 16 if (j == 0 and hp > 0) else 0
                    for _ in range(nbub):
                        nc.tensor.ldweights(dwa[:])
                    for i in range(nkt):
                        s_ = i - 4 * j
                        w0 = 128 * s_ if s_ >= 0 else 0
                        e = etiles.pop(i)
                        if j == 0 and hp > 0:
                            # early-pass steps stall briefly on exp/ezero;
                            # keep the p-state streak alive
                            nc.tensor.ldweights(dwa[:])
                            nc.tensor.ldweights(dwa[:])
                        final_pv = i == nkt - 1
                        is_last_chunk = hp == 3 and j == NJ - 1
                        for u in (0, 1):
                            # software pipeline, lookahead 2: the S-pair for
                            # k-tile i+2 sits between this step's two PVs
                            # (after u0 so the V-tile ldweights hides behind
                            # a streaming matmul on both sides); at i==0 it
                            # goes first so the chunk-entry PSUM wait is
                            # shielded behind it
                            if i + 2 < nkt and u == (1 if i > 0 else 0):
                                emit_s_exp(i + 2)
                            hloc = 2 * hp + u
                            nc.tensor.matmul(
                                pv[u][:, w0:QC],
                                V[i][:, hloc * (HD + 1) : (hloc + 1) * (HD + 1)],
                                e[:, u * QC + w0 : (u + 1) * QC],
                                start=(i == 0),
                                stop=final_pv,
                            )
                            # drain this head's PSUM (accumulator + denom
                            # row) right after its final PV, before u1's
                            # matmul is even emitted: releases the pv bank
                            # ~half a step earlier at every chunk boundary
                            if final_pv and not is_last_chunk:
                                sa = nrmp.tile([HD, QC], f32, tag="sa", name="sa")
                                sd = nrsp.tile([1, QC], f32, tag="sd", name="sd")
                                nc.vector.tensor_copy(sa[:], pv[u][0:HD, :])
                                nc.vector.tensor_copy(sd[:], pv[u][HD : HD + 1, :])
                                sasd[u] = (sa, sd)
                        # deferred norm finishers go first (they unlock AOT
                        # for out chains), on non-diagonal steps only so the
                        # gpsimd broadcast never delays a diagonal ezero
                        if pending_fin and s_ < 0:
                            pending_fin.pop(0)()
                        for fn in sched[hp].pop(step, ()):
                            fn()
                        step += 1
                        # out-proj chains become fillers once unlocked; in the
                        # last head-pair keep 4 in reserve so the PE has work
                        # to chew while the final chunk's normalization runs.
                        # Chains must not be EMITTED before the fins that
                        # write the AOT they read (emission order defines
                        # dependencies), hence the pending_fin guard.
                        if out_ready and not pending_fin and len(out_ready) > 6:
                            out_chain(*out_ready.pop(0))
                    # normalization, split in two: the PSUM drain (copies)
                    # was emitted with the final PV pair above; defer
                    # recip/broadcast/mult into later steps.
                    last = hp == 3 and j == NJ - 1
                    if not last:
                        fins = []
                        for u in (0, 1):
                            sa, sd = sasd[u]

                            def fin(u=u, sa=sa, sd=sd, hp=hp, j=j):
                                rd = nrsp.tile([1, QC], f32, tag="sd", name="rd")
                                nc.vector.reciprocal_approx_fast(rd[:], sd[:])
                                bc = nrmp.tile([HD, QC], f32, tag="sa", name="bc")
                                nc.gpsimd.partition_broadcast(bc[:], rd[:])
                                nc.vector.tensor_tensor(
                                    out=AOT[hp][
                                        64 * u : 64 * u + 64, j * QC : (j + 1) * QC
                                    ],
                                    in0=sa[:],
                                    in1=bc[:],
                                    op=mybir.AluOpType.mult,
                                )

                            fins.append(fin)
                        # flush any leftover finishers of the previous chunk,
                        # then queue this chunk's
                        for fn in pending_fin:
                            fn()
                        pending_fin = fins
                        # after the last head-pair finishes chunk j, its
                        # tokens' output projection is unlocked (the fins
                        # stay deferred: popped in the next chunk's early
                        # steps, before any out chain that reads their AOT)
                        if hp == 3:
                            for tt in range(4 * j, 4 * j + 4):
                                for h in (0, 1):
                                    out_ready.append((tt, h))
                    else:
                        # ---- tail ----
                        # denominator copies split vector/scalar; held-back
                        # chains keep the PE busy during the norm; the mult
                        # reads PSUM directly (no release urgency); the final
                        # 4 token tiles evict into one tile -> a single DMA.
                        sds = {}
                        for u in (0, 1):
                            sd = nrsp.tile([1, QC], f32, tag="sd", name="sd")
                            if u:
                                nc.scalar.copy(sd[:], pv[u][HD : HD + 1, :])
                            else:
                                nc.vector.tensor_copy(sd[:], pv[u][HD : HD + 1, :])
                            sds[u] = sd
                        for fn in pending_fin:
                            fn()
                        pending_fin = []
                        evs = [None, nc.scalar.copy]
                        dqs = [nc.sync, nc.gpsimd]
                        ke = 0
                        while out_ready:
                            out_chain(
                                *out_ready.pop(0),
                                evict=evs[ke % 2],
                                dq=dqs[ke % 2],
                            )
                            ke += 1
                        for u in (0, 1):
                            rd = nrsp.tile([1, QC], f32, tag="sd", name="rd")
                            nc.vector.reciprocal_approx_fast(rd[:], sds[u][:])
                            bc = nrmp.tile([HD, QC], f32, tag="sa", name="bc")
                            nc.gpsimd.partition_broadcast(bc[:], rd[:])
                            nc.vector.tensor_tensor(
                                out=AOT[hp][
                                    64 * u : 64 * u + 64, j * QC : (j + 1) * QC
                                ],
                                in0=pv[u][0:HD, :],
                                in1=bc[:],
                                op=mybir.AluOpType.mult,
                            )
                        for n2, tt2 in enumerate(range(4 * j, 4 * j + 4)):
                            for h in (0, 1):
                                # tail DMAs rotate across the three DMA-
                                # capable queues: the issue instructions
                                # (~0.6us each) no longer serialize on sync
                                out_chain(
                                    tt2, h,
                                    evict=evs[h],
                                    dq=dqs[(2 * n2 + h) % 2],
                                )
                # drain any unconsumed fillers before the next head-pair
                for st_ in sorted(sched[hp]):
                    for fn in sched[hp][st_]:
                        fn()
                sched[hp] = {}
            for fn in pending_fin:
                fn()
            pending_fin = []
            # drain remaining out-proj chains (safety net; normally empty)
            while out_ready:
                out_chain(*out_ready.pop(0))
    nc.compile()
    return nc


def get_program(tok=T):
    if tok not in _prog_cache:
        _prog_cache[tok] = build_program(tok)
    return _prog_cache[tok]


def _pack_pmaj(a, nchunk):
    """[nchunk*128, F] -> [128, nchunk*F] partition-major."""
    F = a.shape[1]
    return np.ascontiguousarray(
        a.reshape(nchunk, 128, F).transpose(1, 0, 2).reshape(128, nchunk * F)
    )


def make_in_maps(x, w_qkv, w_out):
    """Shard full inputs into 8 per-core input maps (bf16, packed layouts)."""
    bf = ml_dtypes.bfloat16
    x = np.asarray(x, dtype=np.float32)
    w_qkv = np.asarray(w_qkv, dtype=np.float32).astype(bf)
    w_out = np.asarray(w_out, dtype=np.float32).astype(bf)
    D = D_MODEL
    # x[b].T partition-major [128, l, tok] then token-chunk-major pieces
    # xT{c} = [128, l, 512] for token chunk c
    xTs = []
    for b in range(x.shape[0]):
        pm = _pack_pmaj(np.ascontiguousarray(x[b].T).astype(bf), 8)  # [128, 8*2048]
        pm = pm.reshape(128, 8, 4, 512)
        xTs.append(
            [np.ascontiguousarray(pm[:, :, c]).reshape(128, 8 * 512) for c in range(4)]
        )
    in_maps = []
    for c in range(N_CORES):
        b, hg = c // 2, c % 2
        m = {}
        for cc in range(4):
            m[f"xT{cc}"] = xTs[b][cc]
        wq = _pack_pmaj(w_qkv[:, hg * FQ : (hg + 1) * FQ], 8)  # [128, l, 512]
        wk = _pack_pmaj(w_qkv[:, D + hg * FQ : D + (hg + 1) * FQ], 8)
        wv = _pack_pmaj(w_qkv[:, 2 * D + hg * FQ : 2 * D + (hg + 1) * FQ], 8)
        for ft in range(4):
            m[f"wq{ft}"] = np.ascontiguousarray(
                wq.reshape(128, 8, 512)[:, :, ft * 128 : (ft + 1) * 128]
            ).reshape(128, 8 * 128)
            m[f"wk{ft}"] = np.ascontiguousarray(
                wk.reshape(128, 8, 512)[:, :, ft * 128 : (ft + 1) * 128]
            ).reshape(128, 8 * 128)
        m["wv0"] = np.ascontiguousarray(wv[:, : 4 * 512])
        m["wv1"] = np.ascontiguousarray(wv[:, 4 * 512 :])
        wo = _pack_pmaj(w_out[hg * FQ : (hg + 1) * FQ, :], 4)  # [128, d, 1024]
        wor = wo.reshape(128, 4, 1024)
        m["wo0"] = np.ascontiguousarray(wor[:, :, :512]).reshape(128, 4 * 512)
        m["wo1"] = np.ascontiguousarray(wor[:, :, 512:]).reshape(128, 4 * 512)
        in_maps.append(m)
    return in_maps


_runner_cache = {}


def _make_runner(nc, n_cores=N_CORES):
    """Cached multi-core executor (same semantics as bass2jax.run_bass_via_pjrt
    for a program with no partition-id and no debug tensors, but the jitted
    callable is reusable so repeat kernel() calls don't recompile)."""
    import jax
    from jax.sharding import Mesh, PartitionSpec
    from jax.experimental.shard_map import shard_map
    import concourse.mybir as mybir
    from concourse.bass2jax import _bass_exec_p, install_neuronx_cc_hook

    install_neuronx_cc_hook()

    in_names, out_names, out_avals = [], [], []
    for alloc in nc.m.functions[0].allocations:
        if not isinstance(alloc, mybir.MemoryLocationSet):
            continue
        name = alloc.memorylocations[0].name
        if alloc.kind == "ExternalInput":
            in_names.append(name)
        elif alloc.kind == "ExternalOutput":
            out_names.append(name)
            out_avals.append(
                jax.core.ShapedArray(
                    tuple(alloc.tensor_shape), mybir.dt.np(alloc.dtype)
                )
            )
    n_params = len(in_names)
    n_outs = len(out_avals)
    all_in_names = in_names + out_names

    def _body(*args):
        outs = _bass_exec_p.bind(
            *args,
            out_avals=tuple(out_avals),
            in_names=tuple(all_in_names),
            out_names=tuple(out_names),
            lowering_input_output_aliases=(),
            sim_require_finite=True,
            sim_require_nnan=True,
            nc=nc,
        )
        return tuple(outs)

    devices = jax.devices()[:n_cores]
    mesh = Mesh(np.asarray(devices), ("core",))
    donate = tuple(range(n_params, n_params + n_outs))
    sharded = jax.jit(
        shard_map(
            _body,
            mesh=mesh,
            in_specs=(PartitionSpec("core"),) * (n_params + n_outs),
            out_specs=(PartitionSpec("core"),) * n_outs,
            check_rep=False,
        ),
        donate_argnums=donate,
        keep_unused=True,
    )

    def run(in_maps):
        per_core = [[np.asarray(m[nm]) for nm in in_names] for m in in_maps]
        concat_in = [
            np.concatenate([per_core[c][i] for c in range(n_cores)], axis=0)
            for i in range(n_params)
        ]
        concat_zeros = [
            np.zeros((n_cores * a.shape[0], *a.shape[1:]), a.dtype)
            for a in out_avals
        ]
        out_arrs = sharded(*concat_in, *concat_zeros)
        return [
            {
                nm: np.asarray(out_arrs[i]).reshape(n_cores, *out_avals[i].shape)[c]
                for i, nm in enumerate(out_names)
            }
            for c in range(n_cores)
        ]

    return run


def get_runner(tok=T):
    if tok not in _runner_cache:
        _runner_cache[tok] = _make_runner(get_program(tok))
    return _runner_cache[tok]


def kernel(x, w_qkv, w_out, b_out):
    in_maps = make_in_maps(x, w_qkv, w_out)
    try:
        run = get_runner(T)
        results = run(in_maps)
    except Exception:
        # fallback: the stock SPMD runner (recompiles per call but is the
        # battle-tested path)
        from concourse.bass_utils import run_bass_kernel_spmd

        results = run_bass_kernel_spmd(
            get_program(T), in_maps, list(range(N_CORES))
        ).results
    b_out = np.asarray(b_out, dtype=np.float32)
    out = np.empty((B, T, D_MODEL), dtype=np.float32)
    for b in range(B):
        out[b] = (
            results[2 * b]["y"].astype(np.float32)
            + results[2 * b + 1]["y"].astype(np.float32)
            + b_out
        )
    return out


# revision 66
# speedup vs baseline: 1.0041x; 1.0041x over previous
"""Causal self-attention TRN2 Bass kernel (bf16, software-pipelined).

Sharding: 8 cores = 4 batches x 2 head-groups. Core c handles batch c//2 and
heads (c%2)*8 .. (c%2)*8+8 (of 16). Each core computes its heads' attention
and a partial output projection; the host sums the two partials per batch and
adds b_out.

Design notes:
  - all matmul operands bf16 (FWL weight loads overlap the stream; f32r
    self-loading matmuls serialize a ~180ns weight load per matmul)
  - DRAM inputs are host-packed into small fine-grained tiles so the first
    S matmul only waits on ~1.5MB (xT is token-chunk-major, weights per-ft)
  - single pool scope, one long instruction stream: QK/V/out projections are
    emitted as deadline-scheduled "filler" half-chains interleaved into the
    attention cadence, so the PE never idles
  - causal mask applied by zeroing exp(S) tiles (affine_select on gpsimd/
    vector), not by adding -inf into PSUM: keeps DVE off the S->exp path
  - softmax denominator via the ones-column of V (row 64 of the PV PSUM);
    normalization split: PSUM drain (copies) immediate, recip/broadcast/mult
    deferred into later steps as filler DVE/gpsimd work
  - warmup matmuls ramp the PE p-state while input DMAs stream

Layouts on chip (per core):
  XTC   4 x [128, 8x512] bf16  x[b].T token-chunk-major: chunk c, d-chunk l
  WQF/WKF 4 x [128, 8x128] bf16 per-ft Q/K weights; WV 2 x [128, 4x512]
  WOH   2 x [128, 4x512] bf16
  QKT   8 x [128, 2048] bf16  Q^T (0..3) / K^T (4..7) features x tokens
  V     16 x [128, 520] bf16  tokens x (8 heads x (64 vals + ones col))
  e     [128, 1024] bf16      exp(S^T) per k-tile, both heads
  AOT   4 x [128, 2048] bf16  normalized attention out (features x tokens)
  y     [2048, 1024] bf16     partial output projection
"""
import sys

sys.path.insert(0, "/opt/trn_rl_repo")

import numpy as np
import ml_dtypes

D_MODEL = 1024
N_HEADS = 16
B = 4
T = 2048
HD = 64
N_CORES = 8
NH_LOC = N_HEADS // 2  # heads per core
FQ = NH_LOC * HD  # 512 local features

_prog_cache = {}


def build_program(tok=T, debug_dumps=False):
    """Build the single-core SPMD Bass program. tok must be a multiple of 512."""
    import concourse.mybir as mybir
    import concourse.tile as tile
    from concourse import bacc

    f32 = mybir.dt.float32
    bf16 = mybir.dt.bfloat16
    P = 128
    QC = 512  # q-chunk width
    KC = D_MODEL // P  # 8 d-model chunks
    TT = tok // P  # token tiles
    NJ = tok // QC  # q-chunks
    NDC = FQ // P  # 4 feature chunks

    nc = bacc.Bacc("TRN2", target_bir_lowering=False, debug=False, num_devices=N_CORES)

    # fine-grained DRAM inputs (host-packed); one tensor per DMA piece so
    # tile-granular dependencies stay small
    xTc = [
        nc.dram_tensor(f"xT{c}", [P, KC * QC], bf16, kind="ExternalInput")
        for c in range(NJ)
    ]
    wqf = [
        nc.dram_tensor(f"wq{ft}", [P, KC * P], bf16, kind="ExternalInput")
        for ft in range(NDC)
    ]
    wkf = [
        nc.dram_tensor(f"wk{ft}", [P, KC * P], bf16, kind="ExternalInput")
        for ft in range(NDC)
    ]
    wvh = [
        nc.dram_tensor(f"wv{h}", [P, 4 * FQ], bf16, kind="ExternalInput")
        for h in range(2)
    ]
    woh = [
        nc.dram_tensor(f"wo{h}", [P, NDC * QC], bf16, kind="ExternalInput")
        for h in range(2)
    ]
    y = nc.dram_tensor("y", [tok, D_MODEL], bf16, kind="ExternalOutput")

    with tile.TileContext(nc) as tc:
        with (
            tc.tile_pool(name="wp", bufs=1) as wp,
            tc.tile_pool(name="xtp", bufs=1) as xtp,
            tc.tile_pool(name="qktp", bufs=1) as qktp,
            tc.tile_pool(name="vp", bufs=1) as vp,
            tc.tile_pool(name="aotp", bufs=1) as aotp,
            tc.tile_pool(name="ep", bufs=3) as ep,
            tc.tile_pool(name="ystp", bufs=4) as ystp,
            tc.tile_pool(name="ybp", bufs=1) as ybp,
            tc.tile_pool(name="mvp", bufs=1) as mvp,
            tc.tile_pool(name="nrm", bufs=6) as nrmp,
            tc.tile_pool(name="nrs", bufs=6) as nrsp,
            tc.tile_pool(name="big", bufs=2, space="PSUM") as bigp,   # 4 banks
            tc.tile_pool(name="pvp", bufs=2, space="PSUM") as pvp,    # 2 banks
            tc.tile_pool(name="prj", bufs=2, space="PSUM") as prjp,   # 2 banks
        ):
            XTC = [wp.tile([P, KC * QC], bf16, tag=f"xtc{c}", name=f"xtc{c}") for c in range(NJ)]
            WQF = [wp.tile([P, KC * P], bf16, tag=f"wqf{ft}", name=f"wqf{ft}") for ft in range(NDC)]
            WKF = [wp.tile([P, KC * P], bf16, tag=f"wkf{ft}", name=f"wkf{ft}") for ft in range(NDC)]
            WVH = [wp.tile([P, 4 * FQ], bf16, tag=f"wvh{h}", name=f"wvh{h}") for h in range(2)]
            WOH = [wp.tile([P, NDC * QC], bf16, tag=f"woh{h}", name=f"woh{h}") for h in range(2)]
            QKT = [qktp.tile([P, tok], bf16, tag=f"qkt{i}", name=f"qkt{i}") for i in range(8)]
            V = [vp.tile([P, NH_LOC * (HD + 1)], bf16, tag=f"v{i}", name=f"v{i}") for i in range(TT)]
            AOT = [aotp.tile([P, tok], bf16, tag=f"aot{d}", name=f"aot{d}") for d in range(NDC)]

            # views
            def xt_view(l, c):  # [128, 512] of d-chunk l, token chunk c
                return XTC[c][:, l * QC : (l + 1) * QC]

            def wq_view(ft, l):
                return WQF[ft][:, l * P : (l + 1) * P]

            def wk_view(ft, l):
                return WKF[ft][:, l * P : (l + 1) * P]

            def wv_view(l):  # [128, 512] all 8 heads' V features, d-chunk l
                return WVH[l // 4][:, (l % 4) * FQ : (l % 4 + 1) * FQ]

            def wo_view(h, d):
                return WOH[h][:, d * QC : (d + 1) * QC]

            # ---------------- input DMAs ----------------
            # three issue queues in parallel, DMA-bandwidth-ordered: the
            # first-S critical set (xtc0 + wq_ft0 + wk_ft0, ~1.5MB) leads,
            # split so no queue serialises more than its share. The xtc
            # pieces are halved (l 0-3 / l 4-7) so region-granular deps let
            # half-chains start as soon as their half lands.
            HX = KC * QC // 2
            nc.gpsimd.dma_start(out=XTC[0][:, :HX], in_=xTc[0][:, :HX])
            nc.sync.dma_start(out=XTC[0][:, HX:], in_=xTc[0][:, HX:])
            nc.scalar.dma_start(out=WQF[0][:], in_=wqf[0][:])
            nc.scalar.dma_start(out=WKF[0][:], in_=wkf[0][:])
            nc.gpsimd.dma_start(out=WVH[0][:], in_=wvh[0][:])
            nc.sync.dma_start(out=WVH[1][:], in_=wvh[1][:])
            nc.gpsimd.dma_start(out=XTC[1][:, :HX], in_=xTc[1][:, :HX])
            nc.sync.dma_start(out=XTC[1][:, HX:], in_=xTc[1][:, HX:])
            nc.scalar.dma_start(out=WQF[1][:], in_=wqf[1][:])
            nc.scalar.dma_start(out=WKF[1][:], in_=wkf[1][:])
            nc.gpsimd.dma_start(out=XTC[2][:, :HX], in_=xTc[2][:, :HX])
            nc.sync.dma_start(out=XTC[2][:, HX:], in_=xTc[2][:, HX:])
            nc.gpsimd.dma_start(out=XTC[3][:, :HX], in_=xTc[3][:, :HX])
            nc.sync.dma_start(out=XTC[3][:, HX:], in_=xTc[3][:, HX:])
            nc.scalar.dma_start(out=WQF[2][:], in_=wqf[2][:])
            nc.scalar.dma_start(out=WKF[2][:], in_=wkf[2][:])
            nc.scalar.dma_start(out=WQF[3][:], in_=wqf[3][:])
            nc.scalar.dma_start(out=WKF[3][:], in_=wkf[3][:])
            nc.sync.dma_start(out=WOH[0][:], in_=woh[0][:])
            nc.sync.dma_start(out=WOH[1][:], in_=woh[1][:])

            # warm the exp table while input DMAs stream
            warm = mvp.tile([1, 8], f32, tag="warm", name="warm")
            nc.vector.memset(warm[:], 0.0)
            nc.scalar.activation(warm[:], warm[:], mybir.ActivationFunctionType.Exp)

            # ones columns of V (value cols are written by the projection
            # eviction; only col 64 of each head group needs initialising).
            # On the vector queue: gpsimd is busy issuing DMAs and must be
            # free early for the first diagonal ezeros.
            for tt in range(TT):
                vv = V[tt][:].rearrange("p (u c) -> p u c", c=HD + 1)
                nc.vector.memset(vv[:, :, HD : HD + 1], 1.0)

            # PE p-state warmup: dummy matmuls with no DMA deps keep the PE
            # "continuously executing" so real matmuls start at full clock
            dwa = mvp.tile([P, P], bf16, tag="dwa", name="dwa")
            dwb = mvp.tile([P, QC], bf16, tag="dwb", name="dwb")
            nc.vector.memset(dwa[:], 0.0)
            nc.vector.memset(dwb[:], 0.0)
            pwarm = prjp.tile([P, QC], f32, tag="prj", name="pwarm")
            for _ in range(16):
                nc.tensor.matmul(pwarm[:, :256], dwa[:], dwb[:, :256], start=True, stop=True)
            # ldweights-only extension: keeps the busy-streak alive through
            # the tail of the input-DMA wait without tying up a PSUM buffer,
            # so the first real matmuls start at the full 2.4GHz p-state
            for _ in range(40):
                nc.tensor.ldweights(dwa[:])

            # ---------------- filler chains (emitted in halves) ----------------
            open_chains = {}

            def qk_half(ft, c, part):
                """QKT[ft][:, c-chunk] = (w-slice)^T @ XT over l; 2 halves."""
                wv_ = wq_view if ft < 4 else wk_view
                fi = ft % 4
                key = ("qk", ft, c)
                if part == 0:
                    open_chains[key] = prjp.tile([P, QC], f32, tag="prj", name=f"pqk{ft}_{c}")
                p = open_chains[key]
                for l in range(4 * part, 4 * part + 4):
                    nc.tensor.matmul(
                        p[:],
                        wv_(fi, l),
                        xt_view(l, c),
                        start=(l == 0),
                        stop=(l == KC - 1),
                    )
                if part == 1:
                    del open_chains[key]
                    nc.vector.tensor_copy(QKT[ft][:, c * QC : (c + 1) * QC], p[:])

            def v_half(tt, part):
                """V[tt] value cols = XT-slice^T @ WV; 2 halves."""
                key = ("v", tt)
                if part == 0:
                    open_chains[key] = prjp.tile([P, FQ], f32, tag="prj", name=f"pv{tt}")
                p = open_chains[key]
                c, s = tt // 4, tt % 4
                for l in range(4 * part, 4 * part + 4):
                    nc.tensor.matmul(
                        p[:],
                        xt_view(l, c)[:, s * P : (s + 1) * P],
                        wv_view(l),
                        start=(l == 0),
                        stop=(l == KC - 1),
                    )
                if part == 1:
                    del open_chains[key]
                    vdst = V[tt][:].rearrange("p (u c) -> p u c", c=HD + 1)[:, :, 0:HD]
                    vsrc = p[:].rearrange("p (u c) -> p u c", c=HD)
                    nc.vector.tensor_copy(vdst, vsrc)

            def out_chain(tt, h, evict=None, dq=None):
                """y[tt-tile, h-half] = AOT-slice^T @ WO, 4 MMs + copy + DMA."""
                p = prjp.tile([P, QC], f32, tag="prj", name=f"py{tt}_{h}")
                for d in range(NDC):
                    nc.tensor.matmul(
                        p[:],
                        AOT[d][:, tt * P : (tt + 1) * P],
                        wo_view(h, d),
                        start=(d == 0),
                        stop=(d == NDC - 1),
                    )
                if evict is None:
                    evict = nc.vector.tensor_copy
                ysb = ystp.tile([P, QC], bf16, tag="y", name="ysb")
                evict(ysb[:], p[:])
                (dq or nc.sync).dma_start(
                    out=y[tt * P : (tt + 1) * P, h * QC : (h + 1) * QC],
                    in_=ysb[:],
                )

            # per-hp filler schedules: {step: [unit, ...]}.  hp0's schedule is
            # deadline-driven (chunk c is read by attention from j=c onward at
            # step 4*c*(c+1)/2...; V[tt] is read by PV at the k-step for tile
            # tt of each j >= tt//4).
            sched = {hp: {} for hp in range(4)}

            def put(hp, step, fn):
                sched[hp].setdefault(step, []).append(fn)

            # hp0, deadline-driven: QK c1 at steps 0-3 (read from step 4),
            # V4-7 at 4-7 (read 8-11), QK c2 at 8-11 (read 12), V8-11 at
            # 12-15 (read 16-19), QK c3 at 16-19 (read 24), V12-15 at 20-23
            # (read 28-31)
            for k, (ft, c) in enumerate([(0, 1), (4, 1)]):
                for part in (0, 1):
                    put(0, 2 * k + part, lambda ft=ft, c=c, part=part: qk_half(ft, c, part))
            for tt in (4, 5, 6, 7):
                for part in (0, 1):
                    put(0, tt, lambda tt=tt, part=part: v_half(tt, part))
            for k, (ft, c) in enumerate([(0, 2), (4, 2)]):
                for part in (0, 1):
                    put(0, 8 + 2 * k + part, lambda ft=ft, c=c, part=part: qk_half(ft, c, part))
            for tt in (8, 9, 10, 11):
                for part in (0, 1):
                    put(0, 4 + tt, lambda tt=tt, part=part: v_half(tt, part))
            for k, (ft, c) in enumerate([(0, 3), (4, 3)]):
                for part in (0, 1):
                    put(0, 16 + 2 * k + part, lambda ft=ft, c=c, part=part: qk_half(ft, c, part))
            for tt in (12, 13, 14, 15):
                for part in (0, 1):
                    put(0, 8 + tt, lambda tt=tt, part=part: v_half(tt, part))
            # QK(hp+1) spread over each hp's steps (hp0's start after its
            # deadline units; hp1/hp2 evenly — ~1 unit per 2.5 steps keeps
            # the per-step PE load under the exp cadence)
            for hp in range(3):
                base = 24 if hp == 0 else 0
                units = []
                for ft in (hp + 1, 4 + hp + 1):
                    for c in range(NJ):
                        for part in (0, 1):
                            units.append(
                                lambda ft=ft, c=c, part=part: qk_half(ft, c, part)
                            )
                # finish by ~80% of the hp's steps so the next head-pair's
                # first S never waits on the last chain's eviction
                span = max(len(units), int(0.8 * (40 - base)))
                for k, fn in enumerate(units):
                    put(hp, base + (k * span) // len(units), fn)

            # ---------------- pre-attention work ----------------
            # half-chain interleave: the QK c0 chains (gating the first S)
            # complete as early as the DMA pieces allow, V chains fill in
            qk_half(0, 0, 0)
            qk_half(4, 0, 0)
            qk_half(0, 0, 1)
            qk_half(4, 0, 1)
            for tt in range(4):
                for part in (0, 1):
                    v_half(tt, part)

            # ---------------- attention + interleaved fillers ----------------
            out_ready = []  # out-proj chains unlocked so far
            pending_fin = []  # deferred normalization finishers
            for hp in range(4):
                step = 0
                for j in range(NJ):
                    nkt = 4 * j + 4
                    pv = {
                        u: pvp.tile([HD + 1, QC], f32, tag="pv", name=f"pv{u}")
                        for u in (0, 1)
                    }
                    etiles = {}
                    sasd = {}

                    def emit_s_exp(i, j=j, etiles=etiles):
                        """S-pair + exp (+ causal zeroing) for k-tile i."""
                        s_ = i - 4 * j
                        w0 = 128 * s_ if s_ >= 0 else 0
                        st = bigp.tile([P, 2 * QC], f32, tag="big", name="st")
                        for u in (0, 1):
                            rs = slice(64 * u, 64 * u + 64)
                            nc.tensor.matmul(
                                st[:, u * QC + w0 : (u + 1) * QC],
                                QKT[4 + hp][rs, i * P : (i + 1) * P],
                                QKT[hp][rs, j * QC + w0 : (j + 1) * QC],
                                start=True,
                                stop=True,
                            )
                        win3 = st[:].rearrange("p (h q) -> p h q", h=2)
                        e = ep.tile([P, 2 * QC], bf16, tag="e", name="e")
                        e3 = e[:].rearrange("p (h q) -> p h q", h=2)

                        def ezero(u):
                            # causal mask: zero exp(S) where q < k in the
                            # diagonal 128x128 block (iota = q_hat - p >= 0
                            # keeps; else fill 0)
                            ev = e[:, u * QC + w0 : u * QC + w0 + P]
                            nc.gpsimd.affine_select(
                                out=ev,
                                in_=ev,
                                compare_op=mybir.AluOpType.is_ge,
                                fill=0.0,
                                base=0,
                                pattern=[[1, P]],
                                channel_multiplier=-1,
                            )

                        nc.scalar.activation(
                            e3[:, :, w0:QC],
                            win3[:, :, w0:QC],
                            mybir.ActivationFunctionType.Exp,
                            scale=0.125,
                        )
                        if s_ >= 0:
                            ezero(0)
                            ezero(1)
                        etiles[i] = e

                    emit_s_exp(0)
                    if nkt > 1:
                        emit_s_exp(1)
                    # p-state bridge: the first PV of a chunk can stall ~1us
                    # on the previous chunk's PSUM release; an idle PE drops
                    # to the 1.2GHz p-state and pays ~3us of half-speed ramp.
                    # No-dep dummy matmuls BEFORE the stalling PV keep the
                    # busy-streak alive through the wait.
                    nbub = 16 if (j == 0 and hp > 0) else 0
                    for _ in range(nbub):
                        nc.tensor.ldweights(dwa[:])
                    for i in range(nkt):
                        s_ = i - 4 * j
                        w0 = 128 * s_ if s_ >= 0 else 0
                        e = etiles.pop(i)
                        if j == 0 and hp > 0:
                            # early-pass steps stall briefly on exp/ezero;
                            # keep the p-state streak alive
                            nc.tensor.ldweights(dwa[:])
                            nc.tensor.ldweights(dwa[:])
                        final_pv = i == nkt - 1
                        is_last_chunk = hp == 3 and j == NJ - 1
                        # software pipeline, lookahead 2: issue S/exp two
                        # k-tiles ahead of this PV so the PE's in-order queue
                        # always has an S-pair between consecutive PVs and
                        # the ACT stream gets a full step of slack
                        if i + 2 < nkt:
                            emit_s_exp(i + 2)
                        for u in (0, 1):
                            hloc = 2 * hp + u
                            nc.tensor.matmul(
                                pv[u][:, w0:QC],
                                V[i][:, hloc * (HD + 1) : (hloc + 1) * (HD + 1)],
                                e[:, u * QC + w0 : (u + 1) * QC],
                                start=(i == 0),
                                stop=final_pv,
                            )
                            # drain this head's PSUM (accumulator + denom
                            # row) right after its final PV, before u1's
                            # matmul is even emitted: releases the pv bank
                            # ~half a step earlier at every chunk boundary
                            if final_pv and not is_last_chunk:
                                sa = nrmp.tile([HD, QC], f32, tag="sa", name="sa")
                                sd = nrsp.tile([1, QC], f32, tag="sd", name="sd")
                                nc.vector.tensor_copy(sa[:], pv[u][0:HD, :])
                                nc.vector.tensor_copy(sd[:], pv[u][HD : HD + 1, :])
                                sasd[u] = (sa, sd)
                        # deferred norm finishers go first (they unlock AOT
                        # for out chains), on non-diagonal steps only so the
                        # gpsimd broadcast never delays a diagonal ezero
                        if pending_fin and s_ < 0:
                            pending_fin.pop(0)()
                        for fn in sched[hp].pop(step, ()):
                            fn()
                        step += 1
                        # out-proj chains become fillers once unlocked; in the
                        # last head-pair keep 4 in reserve so the PE has work
                        # to chew while the final chunk's normalization runs.
                        # Chains must not be EMITTED before the fins that
                        # write the AOT they read (emission order defines
                        # dependencies), hence the pending_fin guard.
                        if out_ready and not pending_fin and len(out_ready) > 6:
                            out_chain(*out_ready.pop(0))
                    # normalization, split in two: the PSUM drain (copies)
                    # was emitted with the final PV pair above; defer
                    # recip/broadcast/mult into later steps.
                    last = hp == 3 and j == NJ - 1
                    if not last:
                        fins = []
                        for u in (0, 1):
                            sa, sd = sasd[u]

                            def fin(u=u, sa=sa, sd=sd, hp=hp, j=j):
                                rd = nrsp.tile([1, QC], f32, tag="sd", name="rd")
                                nc.vector.reciprocal_approx_fast(rd[:], sd[:])
                                bc = nrmp.tile([HD, QC], f32, tag="sa", name="bc")
                                nc.gpsimd.partition_broadcast(bc[:], rd[:])
                                nc.vector.tensor_tensor(
                                    out=AOT[hp][
                                        64 * u : 64 * u + 64, j * QC : (j + 1) * QC
                                    ],
                                    in0=sa[:],
                                    in1=bc[:],
                                    op=mybir.AluOpType.mult,
                                )

                            fins.append(fin)
                        # flush any leftover finishers of the previous chunk,
                        # then queue this chunk's
                        for fn in pending_fin:
                            fn()
                        pending_fin = fins
                        # after the last head-pair finishes chunk j, its
                        # tokens' output projection is unlocked (the fins
                        # stay deferred: popped in the next chunk's early
                        # steps, before any out chain that reads their AOT)
                        if hp == 3:
                            for tt in range(4 * j, 4 * j + 4):
                                for h in (0, 1):
                                    out_ready.append((tt, h))
                    else:
                        # ---- tail ----
                        # denominator copies split vector/scalar; held-back
                        # chains keep the PE busy during the norm; the mult
                        # reads PSUM directly (no release urgency); the final
                        # 4 token tiles evict into one tile -> a single DMA.
                        sds = {}
                        for u in (0, 1):
                            sd = nrsp.tile([1, QC], f32, tag="sd", name="sd")
                            if u:
                                nc.scalar.copy(sd[:], pv[u][HD : HD + 1, :])
                            else:
                                nc.vector.tensor_copy(sd[:], pv[u][HD : HD + 1, :])
                            sds[u] = sd
                        for fn in pending_fin:
                            fn()
                        pending_fin = []
                        evs = [None, nc.scalar.copy]
                        dqs = [nc.sync, nc.gpsimd]
                        ke = 0
                        while out_ready:
                            out_chain(
                                *out_ready.pop(0),
                                evict=evs[ke % 2],
                                dq=dqs[ke % 2],
                            )
                            ke += 1
                        for u in (0, 1):
                            rd = nrsp.tile([1, QC], f32, tag="sd", name="rd")
                            nc.vector.reciprocal_approx_fast(rd[:], sds[u][:])
                            bc = nrmp.tile([HD, QC], f32, tag="sa", name="bc")
                            nc.gpsimd.partition_broadcast(bc[:], rd[:])
                            nc.vector.tensor_tensor(
                                out=AOT[hp][
                                    64 * u : 64 * u + 64, j * QC : (j + 1) * QC
                                ],
                                in0=pv[u][0:HD, :],
                                in1=bc[:],
                                op=mybir.AluOpType.mult,
                            )
                        for n2, tt2 in enumerate(range(4 * j, 4 * j + 4)):
                            for h in (0, 1):
                                # tail DMAs rotate across the three DMA-
                                # capable queues: the issue instructions
                                # (~0.6us each) no longer serialize on sync
                                out_chain(
                                    tt2, h,
                                    evict=evs[h],
                                    dq=dqs[(2 * n2 + h) % 2],
                                )
                # drain any unconsumed fillers before the next head-pair
                for st_ in sorted(sched[hp]):
                    for fn in sched[hp][st_]:
                        fn()
                sched[hp] = {}
            for fn in pending_fin:
                fn()
            pending_fin = []
            # drain remaining out-proj chains (safety net; normally empty)
            while out_ready:
                out_chain(*out_ready.pop(0))
    nc.compile()
    return nc


def get_program(tok=T):
    if tok not in _prog_cache:
        _prog_cache[tok] = build_program(tok)
    return _prog_cache[tok]


def _pack_pmaj(a, nchunk):
    """[nchunk*128, F] -> [128, nchunk*F] partition-major."""
    F = a.shape[1]
    return np.ascontiguousarray(
        a.reshape(nchunk, 128, F).transpose(1, 0, 2).reshape(128, nchunk * F)
    )


def make_in_maps(x, w_qkv, w_out):
    """Shard full inputs into 8 per-core input maps (bf16, packed layouts)."""
    bf = ml_dtypes.bfloat16
    x = np.asarray(x, dtype=np.float32)
    w_qkv = np.asarray(w_qkv, dtype=np.float32).astype(bf)
    w_out = np.asarray(w_out, dtype=np.float32).astype(bf)
    D = D_MODEL
    # x[b].T partition-major [128, l, tok] then token-chunk-major pieces
    # xT{c} = [128, l, 512] for token chunk c
    xTs = []
    for b in range(x.shape[0]):
        pm = _pack_pmaj(np.ascontiguousarray(x[b].T).astype(bf), 8)  # [128, 8*2048]
        pm = pm.reshape(128, 8, 4, 512)
        xTs.append(
            [np.ascontiguousarray(pm[:, :, c]).reshape(128, 8 * 512) for c in range(4)]
        )
    in_maps = []
    for c in range(N_CORES):
        b, hg = c // 2, c % 2
        m = {}
        for cc in range(4):
            m[f"xT{cc}"] = xTs[b][cc]
        wq = _pack_pmaj(w_qkv[:, hg * FQ : (hg + 1) * FQ], 8)  # [128, l, 512]
        wk = _pack_pmaj(w_qkv[:, D + hg * FQ : D + (hg + 1) * FQ], 8)
        wv = _pack_pmaj(w_qkv[:, 2 * D + hg * FQ : 2 * D + (hg + 1) * FQ], 8)
        for ft in range(4):
            m[f"wq{ft}"] = np.ascontiguousarray(
                wq.reshape(128, 8, 512)[:, :, ft * 128 : (ft + 1) * 128]
            ).reshape(128, 8 * 128)
            m[f"wk{ft}"] = np.ascontiguousarray(
                wk.reshape(128, 8, 512)[:, :, ft * 128 : (ft + 1) * 128]
            ).reshape(128, 8 * 128)
        m["wv0"] = np.ascontiguousarray(wv[:, : 4 * 512])
        m["wv1"] = np.ascontiguousarray(wv[:, 4 * 512 :])
        wo = _pack_pmaj(w_out[hg * FQ : (hg + 1) * FQ, :], 4)  # [128, d, 1024]
        wor = wo.reshape(128, 4, 1024)
        m["wo0"] = np.ascontiguousarray(wor[:, :, :512]).reshape(128, 4 * 512)
        m["wo1"] = np.ascontiguousarray(wor[:, :, 512:]).reshape(128, 4 * 512)
        in_maps.append(m)
    return in_maps


_runner_cache = {}


def _make_runner(nc, n_cores=N_CORES):
    """Cached multi-core executor (same semantics as bass2jax.run_bass_via_pjrt
    for a program with no partition-id and no debug tensors, but the jitted
    callable is reusable so repeat kernel() calls don't recompile)."""
    import jax
    from jax.sharding import Mesh, PartitionSpec
    from jax.experimental.shard_map import shard_map
    import concourse.mybir as mybir
    from concourse.bass2jax import _bass_exec_p, install_neuronx_cc_hook

    install_neuronx_cc_hook()

    in_names, out_names, out_avals = [], [], []
    for alloc in nc.m.functions[0].allocations:
        if not isinstance(alloc, mybir.MemoryLocationSet):
            continue
        name = alloc.memorylocations[0].name
        if alloc.kind == "ExternalInput":
            in_names.append(name)
        elif alloc.kind == "ExternalOutput":
            out_names.append(name)
            out_avals.append(
                jax.core.ShapedArray(
                    tuple(alloc.tensor_shape), mybir.dt.np(alloc.dtype)
                )
            )
    n_params = len(in_names)
    n_outs = len(out_avals)
    all_in_names = in_names + out_names

    def _body(*args):
        outs = _bass_exec_p.bind(
            *args,
            out_avals=tuple(out_avals),
            in_names=tuple(all_in_names),
            out_names=tuple(out_names),
            lowering_input_output_aliases=(),
            sim_require_finite=True,
            sim_require_nnan=True,
            nc=nc,
        )
        return tuple(outs)

    devices = jax.devices()[:n_cores]
    mesh = Mesh(np.asarray(devices), ("core",))
    donate = tuple(range(n_params, n_params + n_outs))
    sharded = jax.jit(
        shard_map(
            _body,
            mesh=mesh,
            in_specs=(PartitionSpec("core"),) * (n_params + n_outs),
            out_specs=(PartitionSpec("core"),) * n_outs,
            check_rep=False,
        ),
        donate_argnums=donate,
        keep_unused=True,
    )

    def run(in_maps):
        per_core = [[np.asarray(m[nm]) for nm in in_names] for m in in_maps]
        concat_in = [
            np.concatenate([per_core[c][i] for c in range(n_cores)], axis=0)
            for i in range(n_params)
        ]
        concat_zeros = [
            np.zeros((n_cores * a.shape[0], *a.shape[1:]), a.dtype)
            for a in out_avals
        ]
        out_arrs = sharded(*concat_in, *concat_zeros)
        return [
            {
                nm: np.asarray(out_arrs[i]).reshape(n_cores, *out_avals[i].shape)[c]
                for i, nm in enumerate(out_names)
            }
            for c in range(n_cores)
        ]

    return run


def get_runner(tok=T):
    if tok not in _runner_cache:
        _runner_cache[tok] = _make_runner(get_program(tok))
    return _runner_cache[tok]


def kernel(x, w_qkv, w_out, b_out):
    in_maps = make_in_maps(x, w_qkv, w_out)
    try:
        run = get_runner(T)
        results = run(in_maps)
    except Exception:
        # fallback: the stock SPMD runner (recompiles per call but is the
        # battle-tested path)
        from concourse.bass_utils import run_bass_kernel_spmd

        results = run_bass_kernel_spmd(
            get_program(T), in_maps, list(range(N_CORES))
        ).results
    b_out = np.asarray(b_out, dtype=np.float32)
    out = np.empty((B, T, D_MODEL), dtype=np.float32)
    for b in range(B):
        out[b] = (
            results[2 * b]["y"].astype(np.float32)
            + results[2 * b + 1]["y"].astype(np.float32)
            + b_out
        )
    return out


# revision 67
# speedup vs baseline: 1.0133x; 1.0092x over previous
"""Causal self-attention TRN2 Bass kernel (bf16, software-pipelined).

Sharding: 8 cores = 4 batches x 2 head-groups. Core c handles batch c//2 and
heads (c%2)*8 .. (c%2)*8+8 (of 16). Each core computes its heads' attention
and a partial output projection; the host sums the two partials per batch and
adds b_out.

Design notes:
  - all matmul operands bf16 (FWL weight loads overlap the stream; f32r
    self-loading matmuls serialize a ~180ns weight load per matmul)
  - DRAM inputs are host-packed into small fine-grained tiles so the first
    S matmul only waits on ~1.5MB (xT is token-chunk-major, weights per-ft)
  - single pool scope, one long instruction stream: QK/V/out projections are
    emitted as deadline-scheduled "filler" half-chains interleaved into the
    attention cadence, so the PE never idles
  - causal mask applied by zeroing exp(S) tiles (affine_select on gpsimd/
    vector), not by adding -inf into PSUM: keeps DVE off the S->exp path
  - softmax denominator via the ones-column of V (row 64 of the PV PSUM);
    normalization split: PSUM drain (copies) immediate, recip/broadcast/mult
    deferred into later steps as filler DVE/gpsimd work
  - warmup matmuls ramp the PE p-state while input DMAs stream

Layouts on chip (per core):
  XTC   4 x [128, 8x512] bf16  x[b].T token-chunk-major: chunk c, d-chunk l
  WQF/WKF 4 x [128, 8x128] bf16 per-ft Q/K weights; WV 2 x [128, 4x512]
  WOH   2 x [128, 4x512] bf16
  QKT   8 x [128, 2048] bf16  Q^T (0..3) / K^T (4..7) features x tokens
  V     16 x [128, 520] bf16  tokens x (8 heads x (64 vals + ones col))
  e     [128, 1024] bf16      exp(S^T) per k-tile, both heads
  AOT   4 x [128, 2048] bf16  normalized attention out (features x tokens)
  y     [2048, 1024] bf16     partial output projection
"""
import sys

sys.path.insert(0, "/opt/trn_rl_repo")

import numpy as np
import ml_dtypes

D_MODEL = 1024
N_HEADS = 16
B = 4
T = 2048
HD = 64
N_CORES = 8
NH_LOC = N_HEADS // 2  # heads per core
FQ = NH_LOC * HD  # 512 local features

_prog_cache = {}


def build_program(tok=T, debug_dumps=False):
    """Build the single-core SPMD Bass program. tok must be a multiple of 512."""
    import concourse.mybir as mybir
    import concourse.tile as tile
    from concourse import bacc

    f32 = mybir.dt.float32
    bf16 = mybir.dt.bfloat16
    P = 128
    QC = 512  # q-chunk width
    KC = D_MODEL // P  # 8 d-model chunks
    TT = tok // P  # token tiles
    NJ = tok // QC  # q-chunks
    NDC = FQ // P  # 4 feature chunks

    nc = bacc.Bacc("TRN2", target_bir_lowering=False, debug=False, num_devices=N_CORES)

    # fine-grained DRAM inputs (host-packed); one tensor per DMA piece so
    # tile-granular dependencies stay small
    xTc = [
        nc.dram_tensor(f"xT{c}", [P, KC * QC], bf16, kind="ExternalInput")
        for c in range(NJ)
    ]
    wqf = [
        nc.dram_tensor(f"wq{ft}", [P, KC * P], bf16, kind="ExternalInput")
        for ft in range(NDC)
    ]
    wkf = [
        nc.dram_tensor(f"wk{ft}", [P, KC * P], bf16, kind="ExternalInput")
        for ft in range(NDC)
    ]
    wvh = [
        nc.dram_tensor(f"wv{h}", [P, 4 * FQ], bf16, kind="ExternalInput")
        for h in range(2)
    ]
    woh = [
        nc.dram_tensor(f"wo{h}", [P, NDC * QC], bf16, kind="ExternalInput")
        for h in range(2)
    ]
    y = nc.dram_tensor("y", [tok, D_MODEL], bf16, kind="ExternalOutput")

    with tile.TileContext(nc) as tc:
        with (
            tc.tile_pool(name="wp", bufs=1) as wp,
            tc.tile_pool(name="xtp", bufs=1) as xtp,
            tc.tile_pool(name="qktp", bufs=1) as qktp,
            tc.tile_pool(name="vp", bufs=1) as vp,
            tc.tile_pool(name="aotp", bufs=1) as aotp,
            tc.tile_pool(name="ep", bufs=3) as ep,
            tc.tile_pool(name="ystp", bufs=4) as ystp,
            tc.tile_pool(name="ybp", bufs=1) as ybp,
            tc.tile_pool(name="mvp", bufs=1) as mvp,
            tc.tile_pool(name="nrm", bufs=6) as nrmp,
            tc.tile_pool(name="nrs", bufs=6) as nrsp,
            tc.tile_pool(name="big", bufs=2, space="PSUM") as bigp,   # 4 banks
            tc.tile_pool(name="pvp", bufs=2, space="PSUM") as pvp,    # 2 banks
            tc.tile_pool(name="prj", bufs=2, space="PSUM") as prjp,   # 2 banks
        ):
            XTC = [wp.tile([P, KC * QC], bf16, tag=f"xtc{c}", name=f"xtc{c}") for c in range(NJ)]
            WQF = [wp.tile([P, KC * P], bf16, tag=f"wqf{ft}", name=f"wqf{ft}") for ft in range(NDC)]
            WKF = [wp.tile([P, KC * P], bf16, tag=f"wkf{ft}", name=f"wkf{ft}") for ft in range(NDC)]
            WVH = [wp.tile([P, 4 * FQ], bf16, tag=f"wvh{h}", name=f"wvh{h}") for h in range(2)]
            WOH = [wp.tile([P, NDC * QC], bf16, tag=f"woh{h}", name=f"woh{h}") for h in range(2)]
            QKT = [qktp.tile([P, tok], bf16, tag=f"qkt{i}", name=f"qkt{i}") for i in range(8)]
            V = [vp.tile([P, NH_LOC * (HD + 1)], bf16, tag=f"v{i}", name=f"v{i}") for i in range(TT)]
            AOT = [aotp.tile([P, tok], bf16, tag=f"aot{d}", name=f"aot{d}") for d in range(NDC)]

            # views
            def xt_view(l, c):  # [128, 512] of d-chunk l, token chunk c
                return XTC[c][:, l * QC : (l + 1) * QC]

            def wq_view(ft, l):
                return WQF[ft][:, l * P : (l + 1) * P]

            def wk_view(ft, l):
                return WKF[ft][:, l * P : (l + 1) * P]

            def wv_view(l):  # [128, 512] all 8 heads' V features, d-chunk l
                return WVH[l // 4][:, (l % 4) * FQ : (l % 4 + 1) * FQ]

            def wo_view(h, d):
                return WOH[h][:, d * QC : (d + 1) * QC]

            # ---------------- input DMAs ----------------
            # three issue queues in parallel, DMA-bandwidth-ordered: the
            # first-S critical set (xtc0 + wq_ft0 + wk_ft0, ~1.5MB) leads,
            # split so no queue serialises more than its share. The xtc
            # pieces are halved (l 0-3 / l 4-7) so region-granular deps let
            # half-chains start as soon as their half lands.
            HX = KC * QC // 2
            nc.gpsimd.dma_start(out=XTC[0][:, :HX], in_=xTc[0][:, :HX])
            nc.sync.dma_start(out=XTC[0][:, HX:], in_=xTc[0][:, HX:])
            nc.scalar.dma_start(out=WQF[0][:], in_=wqf[0][:])
            nc.scalar.dma_start(out=WKF[0][:], in_=wkf[0][:])
            nc.gpsimd.dma_start(out=WVH[0][:], in_=wvh[0][:])
            nc.sync.dma_start(out=WVH[1][:], in_=wvh[1][:])
            nc.gpsimd.dma_start(out=XTC[1][:, :HX], in_=xTc[1][:, :HX])
            nc.sync.dma_start(out=XTC[1][:, HX:], in_=xTc[1][:, HX:])
            nc.scalar.dma_start(out=WQF[1][:], in_=wqf[1][:])
            nc.scalar.dma_start(out=WKF[1][:], in_=wkf[1][:])
            nc.gpsimd.dma_start(out=XTC[2][:, :HX], in_=xTc[2][:, :HX])
            nc.sync.dma_start(out=XTC[2][:, HX:], in_=xTc[2][:, HX:])
            nc.gpsimd.dma_start(out=XTC[3][:, :HX], in_=xTc[3][:, :HX])
            nc.sync.dma_start(out=XTC[3][:, HX:], in_=xTc[3][:, HX:])
            nc.scalar.dma_start(out=WQF[2][:], in_=wqf[2][:])
            nc.scalar.dma_start(out=WKF[2][:], in_=wkf[2][:])
            nc.scalar.dma_start(out=WQF[3][:], in_=wqf[3][:])
            nc.scalar.dma_start(out=WKF[3][:], in_=wkf[3][:])
            nc.sync.dma_start(out=WOH[0][:], in_=woh[0][:])
            nc.sync.dma_start(out=WOH[1][:], in_=woh[1][:])

            # warm the exp table while input DMAs stream
            warm = mvp.tile([1, 8], f32, tag="warm", name="warm")
            nc.vector.memset(warm[:], 0.0)
            nc.scalar.activation(warm[:], warm[:], mybir.ActivationFunctionType.Exp)

            # ones columns of V (value cols are written by the projection
            # eviction; only col 64 of each head group needs initialising).
            # On the vector queue: gpsimd is busy issuing DMAs and must be
            # free early for the first diagonal ezeros.
            for tt in range(TT):
                vv = V[tt][:].rearrange("p (u c) -> p u c", c=HD + 1)
                nc.vector.memset(vv[:, :, HD : HD + 1], 1.0)

            # PE p-state warmup: dummy matmuls with no DMA deps keep the PE
            # "continuously executing" so real matmuls start at full clock
            dwa = mvp.tile([P, P], bf16, tag="dwa", name="dwa")
            dwb = mvp.tile([P, QC], bf16, tag="dwb", name="dwb")
            nc.vector.memset(dwa[:], 0.0)
            nc.vector.memset(dwb[:], 0.0)
            pwarm = prjp.tile([P, QC], f32, tag="prj", name="pwarm")
            for _ in range(16):
                nc.tensor.matmul(pwarm[:, :256], dwa[:], dwb[:, :256], start=True, stop=True)
            # ldweights-only extension: keeps the busy-streak alive through
            # the tail of the input-DMA wait without tying up a PSUM buffer,
            # so the first real matmuls start at the full 2.4GHz p-state
            for _ in range(56):
                nc.tensor.ldweights(dwa[:])

            # ---------------- filler chains (emitted in halves) ----------------
            open_chains = {}

            def qk_half(ft, c, part):
                """QKT[ft][:, c-chunk] = (w-slice)^T @ XT over l; 2 halves."""
                wv_ = wq_view if ft < 4 else wk_view
                fi = ft % 4
                key = ("qk", ft, c)
                if part == 0:
                    open_chains[key] = prjp.tile([P, QC], f32, tag="prj", name=f"pqk{ft}_{c}")
                p = open_chains[key]
                for l in range(4 * part, 4 * part + 4):
                    nc.tensor.matmul(
                        p[:],
                        wv_(fi, l),
                        xt_view(l, c),
                        start=(l == 0),
                        stop=(l == KC - 1),
                    )
                if part == 1:
                    del open_chains[key]
                    nc.vector.tensor_copy(QKT[ft][:, c * QC : (c + 1) * QC], p[:])

            def v_half(tt, part):
                """V[tt] value cols = XT-slice^T @ WV; 2 halves."""
                key = ("v", tt)
                if part == 0:
                    open_chains[key] = prjp.tile([P, FQ], f32, tag="prj", name=f"pv{tt}")
                p = open_chains[key]
                c, s = tt // 4, tt % 4
                for l in range(4 * part, 4 * part + 4):
                    nc.tensor.matmul(
                        p[:],
                        xt_view(l, c)[:, s * P : (s + 1) * P],
                        wv_view(l),
                        start=(l == 0),
                        stop=(l == KC - 1),
                    )
                if part == 1:
                    del open_chains[key]
                    vdst = V[tt][:].rearrange("p (u c) -> p u c", c=HD + 1)[:, :, 0:HD]
                    vsrc = p[:].rearrange("p (u c) -> p u c", c=HD)
                    nc.vector.tensor_copy(vdst, vsrc)

            def out_chain(tt, h, evict=None, dq=None):
                """y[tt-tile, h-half] = AOT-slice^T @ WO, 4 MMs + copy + DMA."""
                p = prjp.tile([P, QC], f32, tag="prj", name=f"py{tt}_{h}")
                for d in range(NDC):
                    nc.tensor.matmul(
                        p[:],
                        AOT[d][:, tt * P : (tt + 1) * P],
                        wo_view(h, d),
                        start=(d == 0),
                        stop=(d == NDC - 1),
                    )
                if evict is None:
                    evict = nc.vector.tensor_copy
                ysb = ystp.tile([P, QC], bf16, tag="y", name="ysb")
                evict(ysb[:], p[:])
                (dq or nc.sync).dma_start(
                    out=y[tt * P : (tt + 1) * P, h * QC : (h + 1) * QC],
                    in_=ysb[:],
                )

            # per-hp filler schedules: {step: [unit, ...]}.  hp0's schedule is
            # deadline-driven (chunk c is read by attention from j=c onward at
            # step 4*c*(c+1)/2...; V[tt] is read by PV at the k-step for tile
            # tt of each j >= tt//4).
            sched = {hp: {} for hp in range(4)}

            def put(hp, step, fn):
                sched[hp].setdefault(step, []).append(fn)

            # hp0, deadline-driven: QK c1 at steps 0-3 (read from step 4),
            # V4-7 at 4-7 (read 8-11), QK c2 at 8-11 (read 12), V8-11 at
            # 12-15 (read 16-19), QK c3 at 16-19 (read 24), V12-15 at 20-23
            # (read 28-31)
            for k, (ft, c) in enumerate([(0, 1), (4, 1)]):
                for part in (0, 1):
                    put(0, 2 * k + part, lambda ft=ft, c=c, part=part: qk_half(ft, c, part))
            for tt in (4, 5, 6, 7):
                for part in (0, 1):
                    put(0, tt, lambda tt=tt, part=part: v_half(tt, part))
            for k, (ft, c) in enumerate([(0, 2), (4, 2)]):
                for part in (0, 1):
                    put(0, 8 + 2 * k + part, lambda ft=ft, c=c, part=part: qk_half(ft, c, part))
            for tt in (8, 9, 10, 11):
                for part in (0, 1):
                    put(0, 4 + tt, lambda tt=tt, part=part: v_half(tt, part))
            for k, (ft, c) in enumerate([(0, 3), (4, 3)]):
                for part in (0, 1):
                    put(0, 16 + 2 * k + part, lambda ft=ft, c=c, part=part: qk_half(ft, c, part))
            for tt in (12, 13, 14, 15):
                for part in (0, 1):
                    put(0, 8 + tt, lambda tt=tt, part=part: v_half(tt, part))
            # QK(hp+1) spread over each hp's steps (hp0's start after its
            # deadline units; hp1/hp2 evenly — ~1 unit per 2.5 steps keeps
            # the per-step PE load under the exp cadence)
            for hp in range(3):
                base = 24 if hp == 0 else 0
                units = []
                for ft in (hp + 1, 4 + hp + 1):
                    for c in range(NJ):
                        for part in (0, 1):
                            units.append(
                                lambda ft=ft, c=c, part=part: qk_half(ft, c, part)
                            )
                # finish by ~80% of the hp's steps so the next head-pair's
                # first S never waits on the last chain's eviction
                span = max(len(units), int(0.8 * (40 - base)))
                for k, fn in enumerate(units):
                    put(hp, base + (k * span) // len(units), fn)

            # ---------------- pre-attention work ----------------
            # half-chain interleave: the QK c0 chains (gating the first S)
            # complete as early as the DMA pieces allow, V chains fill in
            qk_half(0, 0, 0)
            qk_half(4, 0, 0)
            qk_half(0, 0, 1)
            qk_half(4, 0, 1)
            for tt in range(4):
                for part in (0, 1):
                    v_half(tt, part)

            # ---------------- attention + interleaved fillers ----------------
            out_ready = []  # out-proj chains unlocked so far
            pending_fin = []  # deferred normalization finishers
            for hp in range(4):
                step = 0
                for j in range(NJ):
                    nkt = 4 * j + 4
                    pv = {
                        u: pvp.tile([HD + 1, QC], f32, tag="pv", name=f"pv{u}")
                        for u in (0, 1)
                    }
                    etiles = {}
                    sasd = {}

                    def emit_s_exp(i, j=j, etiles=etiles):
                        """S-pair + exp (+ causal zeroing) for k-tile i."""
                        s_ = i - 4 * j
                        w0 = 128 * s_ if s_ >= 0 else 0
                        st = bigp.tile([P, 2 * QC], f32, tag="big", name="st")
                        for u in (0, 1):
                            rs = slice(64 * u, 64 * u + 64)
                            nc.tensor.matmul(
                                st[:, u * QC + w0 : (u + 1) * QC],
                                QKT[4 + hp][rs, i * P : (i + 1) * P],
                                QKT[hp][rs, j * QC + w0 : (j + 1) * QC],
                                start=True,
                                stop=True,
                            )
                        win3 = st[:].rearrange("p (h q) -> p h q", h=2)
                        e = ep.tile([P, 2 * QC], bf16, tag="e", name="e")
                        e3 = e[:].rearrange("p (h q) -> p h q", h=2)

                        def ezero(u):
                            # causal mask: zero exp(S) where q < k in the
                            # diagonal 128x128 block (iota = q_hat - p >= 0
                            # keeps; else fill 0)
                            ev = e[:, u * QC + w0 : u * QC + w0 + P]
                            nc.gpsimd.affine_select(
                                out=ev,
                                in_=ev,
                                compare_op=mybir.AluOpType.is_ge,
                                fill=0.0,
                                base=0,
                                pattern=[[1, P]],
                                channel_multiplier=-1,
                            )

                        nc.scalar.activation(
                            e3[:, :, w0:QC],
                            win3[:, :, w0:QC],
                            mybir.ActivationFunctionType.Exp,
                            scale=0.125,
                        )
                        if s_ >= 0:
                            ezero(0)
                            ezero(1)
                        etiles[i] = e

                    emit_s_exp(0)
                    if nkt > 1:
                        emit_s_exp(1)
                    # p-state bridge: the first PV of a chunk can stall ~1us
                    # on the previous chunk's PSUM release; an idle PE drops
                    # to the 1.2GHz p-state and pays ~3us of half-speed ramp.
                    # No-dep dummy matmuls BEFORE the stalling PV keep the
                    # busy-streak alive through the wait.
                    nbub = 16 if (j == 0 and hp > 0) else 0
                    for _ in range(nbub):
                        nc.tensor.ldweights(dwa[:])
                    for i in range(nkt):
                        s_ = i - 4 * j
                        w0 = 128 * s_ if s_ >= 0 else 0
                        e = etiles.pop(i)
                        if j == 0 and hp > 0:
                            # early-pass steps stall briefly on exp/ezero;
                            # keep the p-state streak alive
                            nc.tensor.ldweights(dwa[:])
                            nc.tensor.ldweights(dwa[:])
                        final_pv = i == nkt - 1
                        is_last_chunk = hp == 3 and j == NJ - 1
                        # software pipeline, lookahead 2: issue S/exp two
                        # k-tiles ahead of this PV so the PE's in-order queue
                        # always has an S-pair between consecutive PVs and
                        # the ACT stream gets a full step of slack
                        if i + 2 < nkt:
                            emit_s_exp(i + 2)
                        for u in (0, 1):
                            hloc = 2 * hp + u
                            nc.tensor.matmul(
                                pv[u][:, w0:QC],
                                V[i][:, hloc * (HD + 1) : (hloc + 1) * (HD + 1)],
                                e[:, u * QC + w0 : (u + 1) * QC],
                                start=(i == 0),
                                stop=final_pv,
                            )
                            # drain this head's PSUM (accumulator + denom
                            # row) right after its final PV, before u1's
                            # matmul is even emitted: releases the pv bank
                            # ~half a step earlier at every chunk boundary
                            if final_pv and not is_last_chunk:
                                sa = nrmp.tile([HD, QC], f32, tag="sa", name="sa")
                                sd = nrsp.tile([1, QC], f32, tag="sd", name="sd")
                                nc.vector.tensor_copy(sa[:], pv[u][0:HD, :])
                                nc.vector.tensor_copy(sd[:], pv[u][HD : HD + 1, :])
                                sasd[u] = (sa, sd)
                        # deferred norm finishers go first (they unlock AOT
                        # for out chains), on non-diagonal steps only so the
                        # gpsimd broadcast never delays a diagonal ezero
                        if pending_fin and s_ < 0:
                            pending_fin.pop(0)()
                        for fn in sched[hp].pop(step, ()):
                            fn()
                        step += 1
                        # out-proj chains become fillers once unlocked; in the
                        # last head-pair keep 4 in reserve so the PE has work
                        # to chew while the final chunk's normalization runs.
                        # Chains must not be EMITTED before the fins that
                        # write the AOT they read (emission order defines
                        # dependencies), hence the pending_fin guard.
                        if out_ready and not pending_fin and len(out_ready) > 6:
                            out_chain(*out_ready.pop(0))
                    # normalization, split in two: the PSUM drain (copies)
                    # was emitted with the final PV pair above; defer
                    # recip/broadcast/mult into later steps.
                    last = hp == 3 and j == NJ - 1
                    if not last:
                        fins = []
                        for u in (0, 1):
                            sa, sd = sasd[u]

                            def fin(u=u, sa=sa, sd=sd, hp=hp, j=j):
                                rd = nrsp.tile([1, QC], f32, tag="sd", name="rd")
                                nc.vector.reciprocal_approx_fast(rd[:], sd[:])
                                bc = nrmp.tile([HD, QC], f32, tag="sa", name="bc")
                                nc.gpsimd.partition_broadcast(bc[:], rd[:])
                                nc.vector.tensor_tensor(
                                    out=AOT[hp][
                                        64 * u : 64 * u + 64, j * QC : (j + 1) * QC
                                    ],
                                    in0=sa[:],
                                    in1=bc[:],
                                    op=mybir.AluOpType.mult,
                                )

                            fins.append(fin)
                        # flush any leftover finishers of the previous chunk,
                        # then queue this chunk's
                        for fn in pending_fin:
                            fn()
                        pending_fin = fins
                        # after the last head-pair finishes chunk j, its
                        # tokens' output projection is unlocked (the fins
                        # stay deferred: popped in the next chunk's early
                        # steps, before any out chain that reads their AOT)
                        if hp == 3:
                            for tt in range(4 * j, 4 * j + 4):
                                for h in (0, 1):
                                    out_ready.append((tt, h))
                    else:
                        # ---- tail ----
                        # denominator copies split vector/scalar; held-back
                        # chains keep the PE busy during the norm; the mult
                        # reads PSUM directly (no release urgency); the final
                        # 4 token tiles evict into one tile -> a single DMA.
                        sds = {}
                        for u in (0, 1):
                            sd = nrsp.tile([1, QC], f32, tag="sd", name="sd")
                            if u:
                                nc.scalar.copy(sd[:], pv[u][HD : HD + 1, :])
                            else:
                                nc.vector.tensor_copy(sd[:], pv[u][HD : HD + 1, :])
                            sds[u] = sd
                        for fn in pending_fin:
                            fn()
                        pending_fin = []
                        evs = [None, nc.scalar.copy]
                        dqs = [nc.sync, nc.gpsimd]
                        ke = 0
                        while out_ready:
                            out_chain(
                                *out_ready.pop(0),
                                evict=evs[ke % 2],
                                dq=dqs[ke % 2],
                            )
                            ke += 1
                        for u in (0, 1):
                            rd = nrsp.tile([1, QC], f32, tag="sd", name="rd")
                            nc.vector.reciprocal_approx_fast(rd[:], sds[u][:])
                            bc = nrmp.tile([HD, QC], f32, tag="sa", name="bc")
                            nc.gpsimd.partition_broadcast(bc[:], rd[:])
                            nc.vector.tensor_tensor(
                                out=AOT[hp][
                                    64 * u : 64 * u + 64, j * QC : (j + 1) * QC
                                ],
                                in0=pv[u][0:HD, :],
                                in1=bc[:],
                                op=mybir.AluOpType.mult,
                            )
                        for n2, tt2 in enumerate(range(4 * j, 4 * j + 4)):
                            for h in (0, 1):
                                # tail DMAs rotate across the three DMA-
                                # capable queues: the issue instructions
                                # (~0.6us each) no longer serialize on sync
                                out_chain(
                                    tt2, h,
                                    evict=evs[h],
                                    dq=dqs[(2 * n2 + h) % 2],
                                )
                # drain any unconsumed fillers before the next head-pair
                for st_ in sorted(sched[hp]):
                    for fn in sched[hp][st_]:
                        fn()
                sched[hp] = {}
            for fn in pending_fin:
                fn()
            pending_fin = []
            # drain remaining out-proj chains (safety net; normally empty)
            while out_ready:
                out_chain(*out_ready.pop(0))
    nc.compile()
    return nc


def get_program(tok=T):
    if tok not in _prog_cache:
        _prog_cache[tok] = build_program(tok)
    return _prog_cache[tok]


def _pack_pmaj(a, nchunk):
    """[nchunk*128, F] -> [128, nchunk*F] partition-major."""
    F = a.shape[1]
    return np.ascontiguousarray(
        a.reshape(nchunk, 128, F).transpose(1, 0, 2).reshape(128, nchunk * F)
    )


def make_in_maps(x, w_qkv, w_out):
    """Shard full inputs into 8 per-core input maps (bf16, packed layouts)."""
    bf = ml_dtypes.bfloat16
    x = np.asarray(x, dtype=np.float32)
    w_qkv = np.asarray(w_qkv, dtype=np.float32).astype(bf)
    w_out = np.asarray(w_out, dtype=np.float32).astype(bf)
    D = D_MODEL
    # x[b].T partition-major [128, l, tok] then token-chunk-major pieces
    # xT{c} = [128, l, 512] for token chunk c
    xTs = []
    for b in range(x.shape[0]):
        pm = _pack_pmaj(np.ascontiguousarray(x[b].T).astype(bf), 8)  # [128, 8*2048]
        pm = pm.reshape(128, 8, 4, 512)
        xTs.append(
            [np.ascontiguousarray(pm[:, :, c]).reshape(128, 8 * 512) for c in range(4)]
        )
    in_maps = []
    for c in range(N_CORES):
        b, hg = c // 2, c % 2
        m = {}
        for cc in range(4):
            m[f"xT{cc}"] = xTs[b][cc]
        wq = _pack_pmaj(w_qkv[:, hg * FQ : (hg + 1) * FQ], 8)  # [128, l, 512]
        wk = _pack_pmaj(w_qkv[:, D + hg * FQ : D + (hg + 1) * FQ], 8)
        wv = _pack_pmaj(w_qkv[:, 2 * D + hg * FQ : 2 * D + (hg + 1) * FQ], 8)
        for ft in range(4):
            m[f"wq{ft}"] = np.ascontiguousarray(
                wq.reshape(128, 8, 512)[:, :, ft * 128 : (ft + 1) * 128]
            ).reshape(128, 8 * 128)
            m[f"wk{ft}"] = np.ascontiguousarray(
                wk.reshape(128, 8, 512)[:, :, ft * 128 : (ft + 1) * 128]
            ).reshape(128, 8 * 128)
        m["wv0"] = np.ascontiguousarray(wv[:, : 4 * 512])
        m["wv1"] = np.ascontiguousarray(wv[:, 4 * 512 :])
        wo = _pack_pmaj(w_out[hg * FQ : (hg + 1) * FQ, :], 4)  # [128, d, 1024]
        wor = wo.reshape(128, 4, 1024)
        m["wo0"] = np.ascontiguousarray(wor[:, :, :512]).reshape(128, 4 * 512)
        m["wo1"] = np.ascontiguousarray(wor[:, :, 512:]).reshape(128, 4 * 512)
        in_maps.append(m)
    return in_maps


_runner_cache = {}


def _make_runner(nc, n_cores=N_CORES):
    """Cached multi-core executor (same semantics as bass2jax.run_bass_via_pjrt
    for a program with no partition-id and no debug tensors, but the jitted
    callable is reusable so repeat kernel() calls don't recompile)."""
    import jax
    from jax.sharding import Mesh, PartitionSpec
    from jax.experimental.shard_map import shard_map
    import concourse.mybir as mybir
    from concourse.bass2jax import _bass_exec_p, install_neuronx_cc_hook

    install_neuronx_cc_hook()

    in_names, out_names, out_avals = [], [], []
    for alloc in nc.m.functions[0].allocations:
        if not isinstance(alloc, mybir.MemoryLocationSet):
            continue
        name = alloc.memorylocations[0].name
        if alloc.kind == "ExternalInput":
            in_names.append(name)
        elif alloc.kind == "ExternalOutput":
            out_names.append(name)
            out_avals.append(
                jax.core.ShapedArray(
                    tuple(alloc.tensor_shape), mybir.dt.np(alloc.dtype)
                )
            )
    n_params = len(in_names)
    n_outs = len(out_avals)
    all_in_names = in_names + out_names

    def _body(*args):
        outs = _bass_exec_p.bind(
            *args,
            out_avals=tuple(out_avals),
            in_names=tuple(all_in_names),
            out_names=tuple(out_names),
            lowering_input_output_aliases=(),
            sim_require_finite=True,
            sim_require_nnan=True,
            nc=nc,
        )
        return tuple(outs)

    devices = jax.devices()[:n_cores]
    mesh = Mesh(np.asarray(devices), ("core",))
    donate = tuple(range(n_params, n_params + n_outs))
    sharded = jax.jit(
        shard_map(
            _body,
            mesh=mesh,
            in_specs=(PartitionSpec("core"),) * (n_params + n_outs),
            out_specs=(PartitionSpec("core"),) * n_outs,
            check_rep=False,
        ),
        donate_argnums=donate,
        keep_unused=True,
    )

    def run(in_maps):
        per_core = [[np.asarray(m[nm]) for nm in in_names] for m in in_maps]
        concat_in = [
            np.concatenate([per_core[c][i] for c in range(n_cores)], axis=0)
            for i in range(n_params)
        ]
        concat_zeros = [
            np.zeros((n_cores * a.shape[0], *a.shape[1:]), a.dtype)
            for a in out_avals
        ]
        out_arrs = sharded(*concat_in, *concat_zeros)
        return [
            {
                nm: np.asarray(out_arrs[i]).reshape(n_cores, *out_avals[i].shape)[c]
                for i, nm in enumerate(out_names)
            }
            for c in range(n_cores)
        ]

    return run


def get_runner(tok=T):
    if tok not in _runner_cache:
        _runner_cache[tok] = _make_runner(get_program(tok))
    return _runner_cache[tok]


def kernel(x, w_qkv, w_out, b_out):
    in_maps = make_in_maps(x, w_qkv, w_out)
    try:
        run = get_runner(T)
        results = run(in_maps)
    except Exception:
        # fallback: the stock SPMD runner (recompiles per call but is the
        # battle-tested path)
        from concourse.bass_utils import run_bass_kernel_spmd

        results = run_bass_kernel_spmd(
            get_program(T), in_maps, list(range(N_CORES))
        ).results
    b_out = np.asarray(b_out, dtype=np.float32)
    out = np.empty((B, T, D_MODEL), dtype=np.float32)
    for b in range(B):
        out[b] = (
            results[2 * b]["y"].astype(np.float32)
            + results[2 * b + 1]["y"].astype(np.float32)
            + b_out
        )
    return out
